# revision 47
# baseline (speedup 1.0000x reference)
"""Trainium2 Bass kernel for nn_LrFeatureUpScaler (2-layer TransformerConv GNN).

Sharding over 8 NeuronCores:
  conv1 (4 heads): core i = (head i//2, node-half i%2). Each core computes its
    head's k/v over all nodes, q/skip for its 512 target nodes, in transposed
    [feature, node] layout. One bf16 AllGather shares pre-norm h1 blocks +
    GraphNorm1 partial sums; each core then normalizes full h1 locally.
  conv2 (8 heads): core i = head i; fully local, output in natural
    [node, feature] layout (1/softmax-sum folded into the PSUM extract).
    GraphNorm2 local via ones-matmul stats + partition-broadcast tiles.
  Final row-normalize: per-node partial sum-squares are a second output;
  combined + applied on the host (no second collective).
"""
import numpy as np
import ml_dtypes

import concourse.bass as bass
import concourse.mybir as mybir
import concourse.tile as tile
from concourse.bass_utils import run_bass_kernel_spmd  # noqa: F401
from concourse.masks import make_identity

N = 1024
HR = 2048
EPS = 1e-5
INV_S = float(1.0 / np.sqrt(512.0))
F32 = mybir.dt.float32
BF16 = mybir.dt.bfloat16
AF = mybir.ActivationFunctionType
ALU = mybir.AluOpType
AX = mybir.AxisListType
N_CORES = 8
DEBUG = False

# colpack column layout (f32 [128, CP_W]); vec v of len 128*w stored as
# v.reshape(w, 128).T so column j holds elements j*128..(j+1)*128.
_CP = {}
_o = 0
for _nm, _w in [("bq1", 4), ("bk1", 4), ("bvs1", 4),
                ("gn1g", 16), ("gn1b", 16), ("gn1m", 16),
                ("bq2", 4), ("bk2", 4), ("gn2g", 4), ("gn2b", 4),
                ("gn2m", 4)]:
    _CP[_nm] = (_o, _w)
    _o += _w
CP_W = _o  # 80


def build_nc():
    nc = bass.Bass()
    # ---- I/O ----
    xT = nc.dram_tensor("xT", [N, N], BF16, kind="ExternalInput")
    xt_tgt = nc.dram_tensor("xt_tgt", [N, 512], BF16, kind="ExternalInput")
    x_edge = nc.dram_tensor("x_edge", [512, N], BF16, kind="ExternalInput")
    wq1 = nc.dram_tensor("wq1", [N, 512], BF16, kind="ExternalInput")
    wk1 = nc.dram_tensor("wk1", [N, 512], BF16, kind="ExternalInput")
    wv1 = nc.dram_tensor("wv1", [N, 512], BF16, kind="ExternalInput")
    ws1 = nc.dram_tensor("ws1", [N, 512], BF16, kind="ExternalInput")
    wq2 = nc.dram_tensor("wq2", [HR, 512], BF16, kind="ExternalInput")
    wk2 = nc.dram_tensor("wk2", [HR, 512], BF16, kind="ExternalInput")
    wv2 = nc.dram_tensor("wv2", [HR, 512], BF16, kind="ExternalInput")
    ws2 = nc.dram_tensor("ws2", [HR, 512], BF16, kind="ExternalInput")
    colpack = nc.dram_tensor("colpack", [128, CP_W], F32, kind="ExternalInput")
    colpackb = nc.dram_tensor("colpackb", [128, 8], BF16, kind="ExternalInput")
    # bf16 rows: we1(512) we2(512) bvs2(512)
    rowpack = nc.dram_tensor("rowpack", [1, 1536], BF16, kind="ExternalInput")
    # f32 rows: gn2_gamma(512) gn2_beta(512) gn2_ms(512)
    rowpackf = nc.dram_tensor("rowpackf", [1, 1536], F32, kind="ExternalInput")
    # un-normalized h2 block + GraphNorm2 [scl | sh] rows; the affine and the
    # global row-normalize are applied on the host during unsharding
    out = nc.dram_tensor("out", [N, 512], BF16, kind="ExternalOutput")
    gns = nc.dram_tensor("gns", [1, 1024], F32, kind="ExternalOutput")
    if DEBUG:
        dbg_q = nc.dram_tensor("dbg_q", [128, 2048], BF16, kind="ExternalOutput")
        dbg_k = nc.dram_tensor("dbg_k", [128, 4096], BF16, kind="ExternalOutput")
        dbg_v = nc.dram_tensor("dbg_v", [128, 4096], BF16, kind="ExternalOutput")
        dbg_aT = nc.dram_tensor("dbg_aT", [128, 4096], BF16, kind="ExternalOutput")
        dbg_h1sb = nc.dram_tensor("dbg_h1sb", [128, 2048], BF16, kind="ExternalOutput")
        dbg_h1G = nc.dram_tensor("dbg_h1G", [128, 16448], BF16, kind="ExternalOutput")
        dbg_q2 = nc.dram_tensor("dbg_q2", [128, 4096], BF16, kind="ExternalOutput")
        dbg_h2 = nc.dram_tensor("dbg_h2", [128, 4096], BF16, kind="ExternalOutput")
        dbg_sc = nc.dram_tensor("dbg_sc", [128, 32], F32, kind="ExternalOutput")
        dbg_k2 = nc.dram_tensor("dbg_k2", [128, 4096], BF16, kind="ExternalOutput")
        dbg_al = nc.dram_tensor("dbg_al", [128, 1024], F32, kind="ExternalOutput")
        dbg_E = nc.dram_tensor("dbg_E", [128, 1024], BF16, kind="ExternalOutput")

    with tile.TileContext(nc) as tc:
        with (
            tc.tile_pool(name="const", bufs=1) as cp,
            tc.tile_pool(name="xp", bufs=1) as xp,
            tc.tile_pool(name="w2p", bufs=1) as w2p,
            tc.tile_pool(name="sm", bufs=2) as sm,
            tc.tile_pool(name="smo", bufs=2) as smo,
            tc.tile_pool(name="smE", bufs=2) as smE,
            tc.tile_pool(name="smX", bufs=2) as smX,
            tc.tile_pool(name="smc", bufs=1) as smc,
            tc.tile_pool(name="dram", bufs=1, space="DRAM") as dp,
            tc.tile_pool(name="pp", bufs=4, space="PSUM") as pp,
            tc.tile_pool(name="ppt", bufs=2, space="PSUM") as ppt,
            tc.tile_pool(name="pps", bufs=2, space="PSUM") as pps,
        ):
            def load_w(wt, n_k, nm, pool, g=4):
                t = pool.tile([128, n_k * 512], BF16, name=nm)
                for q in range(n_k // g):
                    nc.sync.dma_start(
                        t[:, q * g * 512:(q + 1) * g * 512]
                        .rearrange("p (k n) -> p k n", n=512),
                        wt[q * g * 128:(q + 1) * g * 128, :]
                        .rearrange("(k p) n -> p k n", p=128))
                return t

            # small-pack tiles (DMAs issued after the first weight loads so
            # their many tiny packets don't delay the first matmuls)
            cpk = cp.tile([128, CP_W], F32, name="cpk")
            cpkb = cp.tile([128, 8], BF16, name="cpkb")
            rpk = cp.tile([1, 1536], BF16, name="rpk")
            rpkf = cp.tile([1, 1536], F32, name="rpkf")

            def load_small_packs():
                nc.sync.dma_start(cpk[:, :], colpack[:, :])
                nc.sync.dma_start(cpkb[:, :], colpackb[:, :])
                nc.sync.dma_start(rpk[0:1, :], rowpack[0:1, :])
                nc.sync.dma_start(rpkf[0:1, :], rowpackf[0:1, :])

            def cpc(nm):
                o, w = _CP[nm]
                return cpk[:, o:o + w]

            we1cb = cpkb[:, 0:4]
            we2cb = cpkb[:, 4:8]
            we1r = rpk[0:1, 0:512]
            we2r = rpk[0:1, 512:1024]
            bvs2r = rpk[0:1, 1024:1536]
            gn2g_r = rpkf[0:1, 0:512]
            gn2b_r = rpkf[0:1, 512:1024]
            gn2m_r = rpkf[0:1, 1024:1536]

            ident = cp.tile([128, 128], F32, name="ident")
            make_identity(nc, ident[:, :])
            identb = cp.tile([128, 128], BF16, name="identb")
            nc.vector.tensor_copy(identb[:, :], ident[:, :])
            # 1/N column: GraphNorm2 stat matmuls produce means directly
            ones_col = cp.tile([128, 1], BF16, name="ones_col")
            nc.gpsimd.memset(ones_col[:, :], 1.0 / N)
            ones_row = cp.tile([1, 128], BF16, name="ones_row")
            nc.gpsimd.memset(ones_row[0:1, :], 1.0)
            eps_col = cp.tile([128, 1], F32, name="eps_col")
            nc.gpsimd.memset(eps_col[:, :], EPS)

            # AllGather split into two halves: AG-a (dc0/dc1 cols) fires
            # mid-conv1; AG-b (dc2/dc3 + stats) at the end. Receive of half a
            # overlaps the wire time of half b.
            ag_in_a = dp.tile([128, 1024], BF16, name="ag_in_a")
            ag_in_b = dp.tile([128, 1032], BF16, name="ag_in_b")
            ag_out_a = dp.tile([8, 128, 1024], BF16, name="ag_out_a",
                               addr_space="Shared")
            ag_out_b = dp.tile([8, 128, 1032], BF16, name="ag_out_b",
                               addr_space="Shared")

            # xT stays resident through conv2 (edge terms)
            xTb = xp.tile([128, 8 * N], BF16, name="xTb")

            def gnorm_coeffs(S1t, S2t, gc, bc, mc, P, w, nm):
                def t(name):
                    return smc.tile([P, w], F32, name=f"{name}{nm}")[0:P, :]
                mu = t("mu")
                nc.vector.tensor_scalar_mul(mu, S1t, 1.0 / N)
                ex2 = t("ex2")
                nc.vector.tensor_scalar_mul(ex2, S2t, 1.0 / N)
                msmu = t("msmu")
                nc.vector.tensor_tensor(msmu, mc, mu, ALU.mult)
                tmp = t("tmp")
                nc.vector.tensor_scalar_mul(tmp, mu, 2.0)
                nc.vector.tensor_tensor(tmp, tmp, msmu, ALU.subtract)
                nc.vector.tensor_tensor(tmp, msmu, tmp, ALU.mult)
                var = t("var")
                nc.vector.tensor_tensor(var, ex2, tmp, ALU.subtract)
                nc.scalar.activation(var, var, AF.Sqrt, bias=eps_col[0:P, 0:1])
                rstd = t("rstd")
                nc.vector.reciprocal(rstd, var)
                scl = t("scl")
                nc.vector.tensor_tensor(scl, gc, rstd, ALU.mult)
                sh = t("sh")
                nc.vector.tensor_tensor(sh, scl, msmu, ALU.mult)
                nc.vector.tensor_tensor(sh, bc, sh, ALU.subtract)
                return scl, sh

            # ================= CONV1 =================
            with tc.tile_pool(name="c1", bufs=1) as c1p:
                # loads ordered by first use: q-proj -> k -> v -> attn -> skip
                xtb = c1p.tile([128, 8 * 512], BF16, name="xtb")
                for q in range(2):
                    nc.sync.dma_start(
                        xtb[:, q * 2048:(q + 1) * 2048]
                        .rearrange("p (k n) -> p k n", n=512),
                        xt_tgt[q * 512:(q + 1) * 512, :]
                        .rearrange("(k p) n -> p k n", p=128))
                w1q = load_w(wq1, 8, "w1q", c1p)
                load_small_packs()
                for q in range(4):
                    nc.sync.dma_start(
                        xTb[:, q * 2 * N:(q + 1) * 2 * N]
                        .rearrange("p (k n) -> p k n", n=N),
                        xT[q * 256:(q + 1) * 256, :]
                        .rearrange("(k p) n -> p k n", p=128))
                w1k = load_w(wk1, 8, "w1k", c1p)
                w1v = load_w(wv1, 8, "w1v", c1p)
                xeb = c1p.tile([128, 4 * N], BF16, name="xeb")
                nc.sync.dma_start(
                    xeb[:, :].rearrange("p (k n) -> p k n", n=N),
                    x_edge.rearrange("(k p) n -> p k n", p=128))
                w1s = load_w(ws1, 8, "w1s", c1p)
                # conv2 q/k weights prefetch (stream in during conv1)
                w2q = load_w(wq2, 16, "w2q", w2p)
                w2k = load_w(wk2, 16, "w2k", w2p)

                qTb = c1p.tile([128, 2048], BF16, name="qTb")
                kTb = c1p.tile([128, 4 * N], BF16, name="kTb")
                v1b = c1p.tile([128, 8 * 512], BF16, name="v1b")
                aT1b = c1p.tile([128, 8 * 512], BF16, name="aT1b")
                # agb = AllGather payload: h1 block (cols 0..2047) + S1/S2 cols
                agb = c1p.tile([128, 2056], BF16, name="agb")

                # qT [512d, 512c]
                for dc in range(4):
                    ps = pp.tile([128, 512], F32, name=f"psq{dc}", tag="mm")
                    for fc in range(8):
                        nc.tensor.matmul(
                            ps[:, :],
                            w1q[:, fc * 512 + dc * 128:fc * 512 + dc * 128 + 128],
                            xtb[:, fc * 512:(fc + 1) * 512],
                            start=(fc == 0), stop=(fc == 7))
                    nc.vector.tensor_scalar(qTb[:, dc * 512:(dc + 1) * 512], ps[:, :],
                                            cpc("bq1")[:, dc:dc + 1], None, ALU.add)
                # kT [512d, 1024r]
                for dc in range(4):
                    for rh in range(2):
                        ps = pp.tile([128, 512], F32, name=f"psk{dc}{rh}", tag="mm")
                        for fc in range(8):
                            nc.tensor.matmul(
                                ps[:, :],
                                w1k[:, fc * 512 + dc * 128:fc * 512 + dc * 128 + 128],
                                xTb[:, fc * N + rh * 512:fc * N + (rh + 1) * 512],
                                start=(fc == 0), stop=(fc == 7))
                        nc.vector.tensor_scalar(
                            kTb[:, dc * N + rh * 512:dc * N + (rh + 1) * 512],
                            ps[:, :], cpc("bk1")[:, dc:dc + 1], None, ALU.add)
                # v natural [1024r, 512d]
                for nk in range(8):
                    ps = pp.tile([128, 512], F32, name=f"psv{nk}", tag="mm")
                    for fc in range(8):
                        nc.tensor.matmul(
                            ps[:, :],
                            xTb[:, fc * N + nk * 128:fc * N + nk * 128 + 128],
                            w1v[:, fc * 512:(fc + 1) * 512],
                            start=(fc == 0), stop=(fc == 7))
                    if nk % 2 == 0:
                        nc.vector.tensor_copy(v1b[:, nk * 512:(nk + 1) * 512], ps[:, :])
                    else:
                        nc.scalar.activation(v1b[:, nk * 512:(nk + 1) * 512],
                                             ps[:, :], AF.Copy)

                # qe[c] = q_c . We
                qe_cols = smc.tile([128, 4], F32, name="qe_cols")
                for cc in range(4):
                    psq = pps.tile([128, 1], F32, name=f"psqe{cc}", tag="sm")
                    for dc in range(4):
                        nc.tensor.matmul(
                            psq[:, :],
                            qTb[:, dc * 512 + cc * 128:dc * 512 + cc * 128 + 128],
                            we1cb[:, dc:dc + 1],
                            start=(dc == 0), stop=(dc == 3))
                    nc.scalar.activation(qe_cols[:, cc:cc + 1], psq[:, :], AF.Copy)

                # softmax per 128-target chunk (no max-subtract); the skip
                # projection for feature chunk dc=cc is emitted between qk
                # and the softmax-dependent tensor ops so the tensor engine
                # stays busy during Exp/vector work.
                skb = c1p.tile([128, 2048], BF16, name="skb")
                t1_cols = smc.tile([128, 4], F32, name="t1_cols")
                for cc in range(4):
                    ps0 = pp.tile([128, 512], F32, name=f"psa{cc}", tag="mm")
                    ps1 = pp.tile([128, 512], F32, name=f"psb{cc}", tag="mm")
                    for dc in range(4):
                        nc.tensor.matmul(
                            ps0[:, :],
                            qTb[:, dc * 512 + cc * 128:dc * 512 + cc * 128 + 128],
                            kTb[:, dc * N:dc * N + 512],
                            start=(dc == 0), stop=(dc == 3))
                    for dc in range(4):
                        nc.tensor.matmul(
                            ps1[:, :],
                            qTb[:, dc * 512 + cc * 128:dc * 512 + cc * 128 + 128],
                            kTb[:, dc * N + 512:dc * N + 1024],
                            start=(dc == 0), stop=(dc == 3))
                    ps_sk = pp.tile([128, 512], F32, name=f"pssk{cc}", tag="mm")
                    for fc in range(8):
                        nc.tensor.matmul(
                            ps_sk[:, :],
                            w1s[:, fc * 512 + cc * 128:fc * 512 + cc * 128 + 128],
                            xtb[:, fc * 512:(fc + 1) * 512],
                            start=(fc == 0), stop=(fc == 7))
                    nc.scalar.activation(skb[:, cc * 512:(cc + 1) * 512],
                                         ps_sk[:, :], AF.Identity,
                                         bias=cpc("bvs1")[:, cc:cc + 1])
                    xe = xeb[:, cc * N:(cc + 1) * N]
                    al = sm.tile([128, N], F32, name=f"al{cc}", tag="al")
                    nc.vector.scalar_tensor_tensor(
                        al[:, 0:512], xe[:, 0:512], qe_cols[:, cc:cc + 1],
                        ps0[:, :], ALU.mult, ALU.add)
                    nc.vector.scalar_tensor_tensor(
                        al[:, 512:1024], xe[:, 512:1024], qe_cols[:, cc:cc + 1],
                        ps1[:, :], ALU.mult, ALU.add)
                    Eb = smE.tile([128, N], BF16, name=f"E{cc}", tag="E")
                    scol = smc.tile([128, 1], F32, name=f"s{cc}")
                    nc.scalar.activation(Eb[:, :], al[:, :], AF.Exp,
                                         scale=float(INV_S), accum_out=scol[:, :])
                    escr = smX.tile([128, N], BF16, name="escr", tag="esc")
                    ucol = smc.tile([128, 1], F32, name=f"u{cc}")
                    nc.vector.scalar_tensor_tensor(
                        escr[:, :], Eb[:, :], 1.0, xe[:, :],
                        ALU.mult, ALU.mult, accum_out=ucol[:, :])
                    rcol = smc.tile([128, 1], F32, name=f"r{cc}")
                    nc.vector.reciprocal(rcol[:, :], scol[:, :])
                    nc.vector.tensor_tensor(t1_cols[:, cc:cc + 1], ucol[:, :],
                                            rcol[:, :], ALU.mult)
                    nc.vector.tensor_scalar_mul(Eb[:, :], Eb[:, :], rcol[:, :])
                    for rc in range(8):
                        pst = ppt.tile([128, 128], BF16, name=f"pt{cc}{rc}", tag="tr")
                        nc.tensor.transpose(pst[:, :], Eb[:, rc * 128:(rc + 1) * 128],
                                            identb[:, :])
                        dst = aT1b[:, rc * 512 + cc * 128:rc * 512 + cc * 128 + 128]
                        if rc % 2 == 0:
                            nc.vector.tensor_copy(dst, pst[:, :])
                        else:
                            nc.scalar.activation(dst, pst[:, :], AF.Copy)
                # t1 row [1, 512] bf16
                t1r = smc.tile([1, 512], BF16, name="t1r")
                pstr = pp.tile([1, 512], F32, name="pst1r", tag="mm")
                for cc in range(4):
                    nc.tensor.transpose(pstr[0:1, cc * 128:(cc + 1) * 128],
                                        t1_cols[:, cc:cc + 1], ident[:, :])
                nc.scalar.activation(t1r[0:1, :], pstr[0:1, :], AF.Copy)

                # h1 block [512f, 512c] = attention + edge term, combined with
                # the pre-computed skip block; fused GraphNorm1 partial sums
                S1c = smc.tile([128, 4], F32, name="S1c")
                S2c = smc.tile([128, 4], F32, name="S2c")
                for dc in range(4):
                    ps = pp.tile([128, 512], F32, name=f"pso{dc}", tag="mm")
                    for rc in range(8):
                        nc.tensor.matmul(
                            ps[:, :],
                            v1b[:, rc * 512 + dc * 128:rc * 512 + dc * 128 + 128],
                            aT1b[:, rc * 512:(rc + 1) * 512],
                            start=(rc == 0), stop=False)
                    nc.tensor.matmul(ps[:, :], we1r[0:1, dc * 128:(dc + 1) * 128],
                                     t1r[0:1, :], start=False, stop=True)
                    nc.vector.scalar_tensor_tensor(
                        agb[:, dc * 512:(dc + 1) * 512], ps[:, :], 1.0,
                        skb[:, dc * 512:(dc + 1) * 512], ALU.mult, ALU.add,
                        accum_out=S1c[:, dc:dc + 1])
                    sq = smX.tile([128, 512], BF16, name=f"sq1_{dc}", tag="esc")
                    nc.scalar.activation(sq[:, :], agb[:, dc * 512:(dc + 1) * 512],
                                         AF.Square, accum_out=S2c[:, dc:dc + 1])
                    # stream each finished 1KB-per-partition column block out
                    # on its own queue as it lands
                    if dc < 2:
                        nc.sync.dma_start(ag_in_a[:, dc * 512:(dc + 1) * 512],
                                          agb[:, dc * 512:(dc + 1) * 512])
                        if dc == 1:
                            nc.gpsimd.collective_compute(
                                "AllGather", ALU.bypass,
                                ins=[ag_in_a.opt()], outs=[ag_out_a.opt()],
                                replica_groups=[list(range(N_CORES))],
                            )
                    elif dc == 2:
                        nc.sync.dma_start(ag_in_b[:, 0:512],
                                          agb[:, 1024:1536])

                nc.vector.tensor_copy(agb[:, 2048:2052], S1c[:, :])
                nc.vector.tensor_copy(agb[:, 2052:2056], S2c[:, :])
                nc.sync.dma_start(ag_in_b[:, 512:1032], agb[:, 1536:2056])
                nc.gpsimd.collective_compute(
                    "AllGather", ALU.bypass,
                    ins=[ag_in_b.opt()], outs=[ag_out_b.opt()],
                    replica_groups=[list(range(N_CORES))],
                )
                if DEBUG:
                    nc.sync.dma_start(dbg_q[:, :], qTb[:, :])
                    nc.sync.dma_start(dbg_k[:, :], kTb[:, :])
                    nc.sync.dma_start(dbg_v[:, :], v1b[:, :])
                    nc.sync.dma_start(dbg_aT[:, :], aT1b[:, :])
                    nc.sync.dma_start(dbg_h1sb[:, :], agb[:, 0:2048])

            # ================= CONV2 scope =================
            with (
                tc.tile_pool(name="c2", bufs=1) as c2p,
                tc.tile_pool(name="atp", bufs=2) as atp,
            ):
                # v/skip weights: SBUF freed by conv1, loads overlap the AG
                w2v = load_w(wv2, 16, "w2v", c2p)
                w2s = load_w(ws2, 16, "w2s", c2p)

                # full h1, group-major: group j = core (head j//2, node-half
                # j%2) occupies cols [j*2056, j*2056+2048) = h1 block
                # [128f-part, 4dc*512n], plus cols +2048..+2056 = that core's
                # GraphNorm1 S1/S2 partial columns.
                h1Gb = c2p.tile([128, 8 * 2056], BF16, name="h1Gb")

                def h1s(fc, ch):
                    o = ((fc // 4) * 2 + ch) * 2056 + (fc % 4) * 512
                    return h1Gb[:, o:o + 512]

                def h1c(fc, nk):
                    o = ((fc // 4) * 2 + nk // 4) * 2056 + (fc % 4) * 512 \
                        + (nk % 4) * 128
                    return h1Gb[:, o:o + 128]

                # assemble full h1 (bf16, group-major): half-a receive DMAs
                # overlap AG-b's wire time; spread across sync/scalar queues
                for j in range(8):
                    eng = nc.sync if j % 2 == 0 else nc.scalar
                    eng.dma_start(
                        h1Gb[:, j * 2056:j * 2056 + 1024],
                        ag_out_a[j, :, :])
                for j in range(8):
                    eng = nc.sync if j % 2 == 0 else nc.scalar
                    eng.dma_start(
                        h1Gb[:, j * 2056 + 1024:(j + 1) * 2056],
                        ag_out_b[j, :, :])
                s1r = [h1Gb[:, j * 2056 + 2048:j * 2056 + 2052] for j in range(8)]
                s2r = [h1Gb[:, j * 2056 + 2052:j * 2056 + 2056] for j in range(8)]
                S1a = smc.tile([128, 16], F32, name="S1a")
                S2a = smc.tile([128, 16], F32, name="S2a")
                for h in range(4):
                    nc.vector.tensor_tensor(S1a[:, 4 * h:4 * h + 4], s1r[2 * h],
                                            s1r[2 * h + 1], ALU.add)
                    nc.vector.tensor_tensor(S2a[:, 4 * h:4 * h + 4], s2r[2 * h],
                                            s2r[2 * h + 1], ALU.add)

                if DEBUG:
                    nc.sync.dma_start(dbg_h1G[:, :], h1Gb[:, :])
                scl1, sh1 = gnorm_coeffs(S1a[:, :], S2a[:, :], cpc("gn1g"),
                                         cpc("gn1b"), cpc("gn1m"), 128, 16, "g1")
                eng = 0
                for g in range(2):
                    for f in range(16):
                        o = ((f // 4) * 2 + g) * 2056 + (f % 4) * 512
                        dst = h1Gb[:, o:o + 512]
                        if eng % 3 == 0:
                            nc.vector.tensor_scalar(dst, dst, scl1[:, f:f + 1],
                                                    sh1[:, f:f + 1], ALU.mult, ALU.add)
                        elif eng % 3 == 1:
                            nc.scalar.activation(dst, dst, AF.Identity,
                                                 bias=sh1[:, f:f + 1],
                                                 scale=scl1[:, f:f + 1])
                        else:
                            nc.gpsimd.tensor_scalar(dst, dst, scl1[:, f:f + 1],
                                                    sh1[:, f:f + 1], ALU.mult, ALU.add)
                        eng += 1

                # ---- conv2 compute (natural-layout output) ----
                q2Tb = c2p.tile([128, 4 * N], BF16, name="q2Tb")
                k2Tb = c2p.tile([128, 4 * N], BF16, name="k2Tb")
                v2b = c2p.tile([128, 8 * 512], BF16, name="v2b")
                h2nb = c2p.tile([128, 8 * 512], BF16, name="h2nb")
                rcols = smc.tile([128, 8], F32, name="rcols")

                for dc in range(4):
                    for ch in range(2):
                        ps = pp.tile([128, 512], F32, name=f"ps2q{dc}{ch}", tag="mm")
                        for fc in range(16):
                            nc.tensor.matmul(
                                ps[:, :],
                                w2q[:, fc * 512 + dc * 128:fc * 512 + dc * 128 + 128],
                                h1s(fc, ch), start=(fc == 0), stop=(fc == 15))
                        nc.vector.tensor_scalar(
                            q2Tb[:, dc * N + ch * 512:dc * N + (ch + 1) * 512],
                            ps[:, :], cpc("bq2")[:, dc:dc + 1], None, ALU.add)
                for dc in range(4):
                    for ch in range(2):
                        ps = pp.tile([128, 512], F32, name=f"ps2k{dc}{ch}", tag="mm")
                        for fc in range(16):
                            nc.tensor.matmul(
                                ps[:, :],
                                w2k[:, fc * 512 + dc * 128:fc * 512 + dc * 128 + 128],
                                h1s(fc, ch), start=(fc == 0), stop=(fc == 15))
                        nc.vector.tensor_scalar(
                            k2Tb[:, dc * N + ch * 512:dc * N + (ch + 1) * 512],
                            ps[:, :], cpc("bk2")[:, dc:dc + 1], None, ALU.add)
                if DEBUG:
                    nc.sync.dma_start(dbg_k2[:, :], k2Tb[:, :])
                for nk in range(8):
                    ps = pp.tile([128, 512], F32, name=f"ps2v{nk}", tag="mm")
                    for fc in range(16):
                        nc.tensor.matmul(ps[:, :], h1c(fc, nk),
                                         w2v[:, fc * 512:(fc + 1) * 512],
                                         start=(fc == 0), stop=(fc == 15))
                    if nk % 2 == 0:
                        nc.vector.tensor_copy(v2b[:, nk * 512:(nk + 1) * 512], ps[:, :])
                    else:
                        nc.scalar.activation(v2b[:, nk * 512:(nk + 1) * 512],
                                             ps[:, :], AF.Copy)

                qe2 = smc.tile([128, 8], F32, name="qe2")
                for cc in range(8):
                    psq = pps.tile([128, 1], F32, name=f"ps2e{cc}", tag="sm")
                    for dc in range(4):
                        nc.tensor.matmul(
                            psq[:, :],
                            q2Tb[:, dc * N + cc * 128:dc * N + cc * 128 + 128],
                            we2cb[:, dc:dc + 1], start=(dc == 0), stop=(dc == 3))
                    nc.scalar.activation(qe2[:, cc:cc + 1], psq[:, :], AF.Copy)

                if DEBUG:
                    nc.sync.dma_start(dbg_q2[:, :], q2Tb[:, :])
                if DEBUG:
                    nc.sync.dma_start(dbg_sc[:, 8:16], qe2[:, :])
                t2rr = smc.tile([1, N], BF16, name="t2rr")
                scols_dbg = smc.tile([128, 8], F32, name="scols_dbg")

                sq2b = c2p.tile([128, 8 * 512], BF16, name="sq2b")
                for cc in range(8):
                    ps0 = pp.tile([128, 512], F32, name=f"p2a{cc}", tag="mm")
                    ps1 = pp.tile([128, 512], F32, name=f"p2b{cc}", tag="mm")
                    for dc in range(4):
                        nc.tensor.matmul(
                            ps0[:, :],
                            q2Tb[:, dc * N + cc * 128:dc * N + cc * 128 + 128],
                            k2Tb[:, dc * N:dc * N + 512],
                            start=(dc == 0), stop=(dc == 3))
                    for dc in range(4):
                        nc.tensor.matmul(
                            ps1[:, :],
                            q2Tb[:, dc * N + cc * 128:dc * N + cc * 128 + 128],
                            k2Tb[:, dc * N + 512:dc * N + 1024],
                            start=(dc == 0), stop=(dc == 3))
                    # skip projection emitted here: fills the tensor engine
                    # while the softmax chain runs on vector/scalar
                    ps_skp = pp.tile([128, 512], F32, name=f"psk2{cc}", tag="mm")
                    for fc in range(16):
                        nc.tensor.matmul(ps_skp[:, :], h1c(fc, cc),
                                         w2s[:, fc * 512:(fc + 1) * 512],
                                         start=(fc == 0), stop=False)
                    nc.tensor.matmul(ps_skp[:, :], ones_row[0:1, :], bvs2r[0:1, :],
                                     start=False, stop=True)
                    skp_sb = smo.tile([128, 512], F32, name=f"skp{cc}", tag="of")
                    nc.scalar.activation(skp_sb[:, :], ps_skp[:, :], AF.Copy)
                    xe = xTb[:, cc * N:(cc + 1) * N]
                    al = sm.tile([128, N], F32, name=f"al2_{cc}", tag="al")
                    nc.vector.scalar_tensor_tensor(
                        al[:, 0:512], xe[:, 0:512], qe2[:, cc:cc + 1],
                        ps0[:, :], ALU.mult, ALU.add)
                    nc.vector.scalar_tensor_tensor(
                        al[:, 512:1024], xe[:, 512:1024], qe2[:, cc:cc + 1],
                        ps1[:, :], ALU.mult, ALU.add)
                    Eb = smE.tile([128, N], BF16, name=f"E2_{cc}", tag="E")
                    scol = smc.tile([128, 1], F32, name=f"s2_{cc}")
                    nc.scalar.activation(Eb[:, :], al[:, :], AF.Exp,
                                         scale=float(INV_S), accum_out=scol[:, :])
                    escr = smX.tile([128, N], BF16, name="escr2", tag="esc")
                    ucol = smc.tile([128, 1], F32, name=f"u2_{cc}")
                    nc.vector.scalar_tensor_tensor(
                        escr[:, :], Eb[:, :], 1.0, xe[:, :],
                        ALU.mult, ALU.mult, accum_out=ucol[:, :])
                    nc.vector.reciprocal(rcols[:, cc:cc + 1], scol[:, :])
                    if DEBUG:
                        nc.vector.tensor_copy(scols_dbg[:, cc:cc + 1], scol[:, :])
                        if cc == 0:
                            nc.sync.dma_start(dbg_al[:, :], al[:, :])
                            nc.sync.dma_start(dbg_E[:, :], Eb[:, :])
                    pstt = pps.tile([1, 128], F32, name=f"ptt{cc}", tag="sm")
                    nc.tensor.transpose(pstt[0:1, :], ucol[:, :], ident[:, :])
                    nc.scalar.activation(t2rr[0:1, cc * 128:(cc + 1) * 128],
                                         pstt[0:1, :], AF.Copy)
                    # transpose E into the per-chunk aT ring
                    aTc = atp.tile([128, 8 * 128], BF16, name=f"aTc{cc}", tag="at")
                    for rc in range(8):
                        pst = ppt.tile([128, 128], BF16, name=f"p2t{cc}{rc}", tag="tr")
                        nc.tensor.transpose(pst[:, :], Eb[:, rc * 128:(rc + 1) * 128],
                                            identb[:, :])
                        dst = aTc[:, rc * 128:(rc + 1) * 128]
                        if rc % 2 == 0:
                            nc.vector.tensor_copy(dst, pst[:, :])
                        else:
                            nc.scalar.activation(dst, pst[:, :], AF.Copy)

                    # output chunk cc: natural layout [128c, 512d]
                    ps_att = pp.tile([128, 512], F32, name=f"pat{cc}", tag="mm")
                    for rc in range(8):
                        nc.tensor.matmul(
                            ps_att[:, :], aTc[:, rc * 128:(rc + 1) * 128],
                            v2b[:, rc * 512:(rc + 1) * 512],
                            start=(rc == 0), stop=False)
                    nc.tensor.matmul(ps_att[:, :],
                                     t2rr[0:1, cc * 128:(cc + 1) * 128],
                                     we2r[0:1, :], start=False, stop=True)
                    nc.vector.scalar_tensor_tensor(
                        h2nb[:, cc * 512:(cc + 1) * 512], ps_att[:, :],
                        rcols[:, cc:cc + 1], skp_sb[:, :], ALU.mult, ALU.add)
                    # stream the raw h2 chunk out now (hidden under the loop)
                    nc.sync.dma_start(out[cc * 128:(cc + 1) * 128, :],
                                      h2nb[:, cc * 512:(cc + 1) * 512])
                    # square for GraphNorm2 E[x^2], computed as chunks land
                    sqd = sq2b[:, cc * 512:(cc + 1) * 512]
                    if cc % 2 == 0:
                        nc.gpsimd.tensor_tensor(
                            sqd, h2nb[:, cc * 512:(cc + 1) * 512],
                            h2nb[:, cc * 512:(cc + 1) * 512], ALU.mult)
                    else:
                        nc.scalar.activation(sqd,
                                             h2nb[:, cc * 512:(cc + 1) * 512],
                                             AF.Square)

                # GraphNorm2 stats via 1/N-matmuls -> means directly
                S1ps = pp.tile([1, 512], F32, name="S1ps", tag="mm")
                S2ps = pp.tile([1, 512], F32, name="S2ps", tag="mm")
                for cc in range(8):
                    nc.tensor.matmul(S1ps[0:1, :], ones_col[:, 0:1],
                                     h2nb[:, cc * 512:(cc + 1) * 512],
                                     start=(cc == 0), stop=(cc == 7))
                for cc in range(8):
                    nc.tensor.matmul(S2ps[0:1, :], ones_col[:, 0:1],
                                     sq2b[:, cc * 512:(cc + 1) * 512],
                                     start=(cc == 0), stop=(cc == 7))

                # GraphNorm2 coeffs computed directly in row space [1, 512];
                # S1ps/S2ps already hold mu and E[x^2] (1/N-weighted matmul).
                # rA = msmu then sh; rB = tmp -> var -> std -> scl (in-place)
                def rt(name):
                    return smc.tile([1, 512], F32, name=name)[0:1, :]
                rA = rt("rA")
                nc.vector.tensor_tensor(rA, gn2m_r, S1ps[0:1, :], ALU.mult)
                rB = rt("rB")
                nc.vector.scalar_tensor_tensor(
                    rB, S1ps[0:1, :], 2.0, rA, ALU.mult, ALU.subtract)
                nc.vector.tensor_tensor(rB, rA, rB, ALU.mult)
                nc.vector.tensor_tensor(rB, S2ps[0:1, :], rB, ALU.subtract)
                nc.scalar.activation(rB, rB, AF.Sqrt, bias=eps_col[0:1, 0:1])
                rC = rt("rC")
                nc.vector.reciprocal(rC, rB)
                nc.vector.tensor_tensor(rB, gn2g_r, rC, ALU.mult)
                nc.vector.tensor_tensor(rA, rB, rA, ALU.mult)
                nc.vector.tensor_tensor(rC, gn2b_r, rA, ALU.subtract)
                nc.sync.dma_start(gns[0:1, 0:512], rB)
                nc.sync.dma_start(gns[0:1, 512:1024], rC)
    return nc


_NC_CACHE = None


def _get_nc():
    global _NC_CACHE
    if _NC_CACHE is None:
        nc = build_nc()
        # local walrus only accepts one sync-wait per CTRL-class instruction
        for f in nc.m.functions:
            for bb in f.blocks:
                changed = False
                new_list = []
                for ins in bb.instructions:
                    si = ins.sync_info
                    if si is not None and len(si.on_wait) > 1:
                        waits = list(si.on_wait)
                        for i, w in enumerate(waits[:-1]):
                            nop = mybir.InstNoOp(
                                name=f"{ins.name}_presplit{i}", engine=ins.engine)
                            nop.sync_info = mybir.SyncInfo(on_wait=[w], on_update=[])
                            new_list.append(nop)
                        ins.sync_info = mybir.SyncInfo(
                            on_wait=[waits[-1]], on_update=list(si.on_update))
                        changed = True
                    new_list.append(ins)
                if changed:
                    bb.instructions = new_list
        _NC_CACHE = nc
    return _NC_CACHE


def _col(v, w):
    """[128*w] vector -> [128, w] column layout (col j = elems j*128..+128)."""
    return np.ascontiguousarray(np.asarray(v, np.float32).reshape(w, 128).T)


def build_in_maps(inputs):
    x = np.asarray(inputs["x"], np.float32)
    bf = ml_dtypes.bfloat16

    def c(a, dt=np.float32):
        return np.ascontiguousarray(a).astype(dt)

    f32 = {k: np.asarray(v, np.float32) for k, v in inputs.items() if k != "x"}
    xT = np.ascontiguousarray(x.T)
    in_maps = []
    for i in range(N_CORES):
        h, g = i // 2, i % 2
        s1, s2i = slice(512 * h, 512 * (h + 1)), slice(512 * i, 512 * (i + 1))
        we1h = f32["e1_w"].reshape(4, 512)[h]
        we2h = f32["e2_w"].reshape(8, 512)[i]
        bvs1 = f32["v1_b"][s1] + f32["s1_b"][s1]
        bvs2 = f32["v2_b"][s2i] + f32["s2_b"][s2i]
        colpack = np.concatenate([
            _col(f32["q1_b"][s1], 4), _col(f32["k1_b"][s1], 4), _col(bvs1, 4),
            _col(f32["gn1_gamma"], 16), _col(f32["gn1_beta"], 16),
            _col(f32["gn1_ms"], 16),
            _col(f32["q2_b"][s2i], 4), _col(f32["k2_b"][s2i], 4),
            _col(f32["gn2_gamma"][s2i], 4), _col(f32["gn2_beta"][s2i], 4),
            _col(f32["gn2_ms"][s2i], 4),
        ], axis=1)
        colpackb = np.concatenate([_col(we1h, 4), _col(we2h, 4)], axis=1)
        rowpack = np.concatenate([we1h, we2h, bvs2]).reshape(1, 1536)
        rowpackf = np.concatenate([
            f32["gn2_gamma"][s2i], f32["gn2_beta"][s2i], f32["gn2_ms"][s2i],
        ]).reshape(1, 1536)
        m = {
            "xT": c(xT, bf),
            "xt_tgt": c(xT[:, 512 * g:512 * (g + 1)], bf),
            "x_edge": c(xT[512 * g:512 * (g + 1), :], bf),
            "wq1": c(f32["q1_w"][:, s1], bf),
            "wk1": c(f32["k1_w"][:, s1], bf),
            "wv1": c(f32["v1_w"][:, s1], bf),
            "ws1": c(f32["s1_w"][:, s1], bf),
            "wq2": c(f32["q2_w"][:, s2i], bf),
            "wk2": c(f32["k2_w"][:, s2i], bf),
            "wv2": c(f32["v2_w"][:, s2i], bf),
            "ws2": c(f32["s2_w"][:, s2i], bf),
            "colpack": c(colpack),
            "colpackb": c(colpackb, bf),
            "rowpack": c(rowpack, bf),
            "rowpackf": c(rowpackf),
        }
        in_maps.append(m)
    return in_maps


def kernel(**inputs):
    in_maps = build_in_maps(inputs)
    res = _run_cached(in_maps)
    full = np.empty((N, 2 * HR), np.float32)
    for i in range(N_CORES):
        g = np.asarray(res[i]["gns"], np.float32).reshape(1024)
        h2 = np.asarray(res[i]["out"], np.float32)
        full[:, 512 * i:512 * (i + 1)] = h2 * g[:512] + g[512:]
    rnsum = (full.astype(np.float64) ** 2).sum(axis=1)
    full /= np.sqrt(rnsum)[:, None].astype(np.float32)
    return full


_RUNNER = None


def _get_runner():
    """Build the sharded jitted executable once per process."""
    global _RUNNER
    if _RUNNER is not None:
        return _RUNNER
    import jax
    from jax.sharding import Mesh, PartitionSpec, NamedSharding
    from jax.experimental.shard_map import shard_map
    from concourse import bass2jax
    from concourse.bass2jax import _bass_exec_p, install_neuronx_cc_hook

    nc = _get_nc()
    install_neuronx_cc_hook()
    partition_name = nc.partition_id_tensor.name if nc.partition_id_tensor else None
    in_names, out_names, out_avals, zero_outs = [], [], [], []
    for alloc in nc.m.functions[0].allocations:
        if not isinstance(alloc, mybir.MemoryLocationSet):
            continue
        name = alloc.memorylocations[0].name
        if alloc.kind == "ExternalInput":
            if name != partition_name:
                in_names.append(name)
        elif alloc.kind == "ExternalOutput":
            out_names.append(name)
            out_avals.append(jax.core.ShapedArray(
                tuple(alloc.tensor_shape), mybir.dt.np(alloc.dtype)))
            zero_outs.append(np.zeros(tuple(alloc.tensor_shape),
                                      mybir.dt.np(alloc.dtype)))
    n_params, n_outs = len(in_names), len(out_avals)
    all_names = in_names + out_names + ([partition_name] if partition_name else [])
    donate = tuple(range(n_params, n_params + n_outs))

    def _body(*args):
        operands = list(args)
        if partition_name is not None:
            operands.append(bass2jax.partition_id_tensor())
        return tuple(_bass_exec_p.bind(
            *operands, out_avals=tuple(out_avals), in_names=tuple(all_names),
            out_names=tuple(out_names), lowering_input_output_aliases=(),
            sim_require_finite=True, sim_require_nnan=True, nc=nc))

    devices = jax.devices()[:N_CORES]
    mesh = Mesh(np.asarray(devices), ("core",))
    sharded = jax.jit(
        shard_map(_body, mesh=mesh,
                  in_specs=(PartitionSpec("core"),) * (n_params + n_outs),
                  out_specs=(PartitionSpec("core"),) * n_outs,
                  check_rep=False),
        donate_argnums=donate, keep_unused=True)
    sh = NamedSharding(mesh, PartitionSpec("core"))
    _RUNNER = (sharded, sh, in_names, out_names, out_avals, zero_outs, jax)
    return _RUNNER


def _run_cached(in_maps):
    sharded, sh, in_names, out_names, out_avals, zero_outs, jax = _get_runner()
    concat_in = [np.concatenate([np.asarray(in_maps[c][nm])
                                 for c in range(N_CORES)], axis=0)
                 for nm in in_names]
    dev_in = [jax.device_put(a, sh) for a in concat_in]
    zs = [jax.device_put(np.zeros((N_CORES * z.shape[0], *z.shape[1:]), z.dtype), sh)
          for z in zero_outs]
    outs = sharded(*dev_in, *zs)
    outs = [np.asarray(o).reshape(N_CORES, *out_avals[i].shape)
            for i, o in enumerate(outs)]
    return [{nm: outs[i][c] for i, nm in enumerate(out_names)}
            for c in range(N_CORES)]



# revision 50
# speedup vs baseline: 1.0231x; 1.0231x over previous
"""Trainium2 Bass kernel for nn_LrFeatureUpScaler (2-layer TransformerConv GNN).

Sharding over 8 NeuronCores:
  conv1 (4 heads): core i = (head i//2, node-half i%2). Each core computes its
    head's k/v over all nodes, q/skip for its 512 target nodes, in transposed
    [feature, node] layout. One bf16 AllGather shares pre-norm h1 blocks +
    GraphNorm1 partial sums; each core then normalizes full h1 locally.
  conv2 (8 heads): core i = head i; fully local, output in natural
    [node, feature] layout (1/softmax-sum folded into the PSUM extract).
    GraphNorm2 local via ones-matmul stats + partition-broadcast tiles.
  Final row-normalize: per-node partial sum-squares are a second output;
  combined + applied on the host (no second collective).
"""
import numpy as np
import ml_dtypes

import concourse.bass as bass
import concourse.mybir as mybir
import concourse.tile as tile
from concourse.bass_utils import run_bass_kernel_spmd  # noqa: F401
from concourse.masks import make_identity

N = 1024
HR = 2048
EPS = 1e-5
INV_S = float(1.0 / np.sqrt(512.0))
F32 = mybir.dt.float32
BF16 = mybir.dt.bfloat16
AF = mybir.ActivationFunctionType
ALU = mybir.AluOpType
AX = mybir.AxisListType
N_CORES = 8
DEBUG = False

# colpack column layout (f32 [128, CP_W]); vec v of len 128*w stored as
# v.reshape(w, 128).T so column j holds elements j*128..(j+1)*128.
_CP = {}
_o = 0
for _nm, _w in [("bq1", 4), ("bk1", 4), ("bvs1", 4),
                ("gn1g", 16), ("gn1b", 16), ("gn1m", 16),
                ("bq2", 4), ("bk2", 4), ("gn2g", 4), ("gn2b", 4),
                ("gn2m", 4)]:
    _CP[_nm] = (_o, _w)
    _o += _w
CP_W = _o  # 80


def build_nc():
    nc = bass.Bass()
    # ---- I/O ----
    xT = nc.dram_tensor("xT", [N, N], BF16, kind="ExternalInput")
    xt_tgt = nc.dram_tensor("xt_tgt", [N, 512], BF16, kind="ExternalInput")
    x_edge = nc.dram_tensor("x_edge", [512, N], BF16, kind="ExternalInput")
    wq1 = nc.dram_tensor("wq1", [N, 512], BF16, kind="ExternalInput")
    wk1 = nc.dram_tensor("wk1", [N, 512], BF16, kind="ExternalInput")
    wv1 = nc.dram_tensor("wv1", [N, 512], BF16, kind="ExternalInput")
    ws1 = nc.dram_tensor("ws1", [N, 512], BF16, kind="ExternalInput")
    wq2 = nc.dram_tensor("wq2", [HR, 512], BF16, kind="ExternalInput")
    wk2 = nc.dram_tensor("wk2", [HR, 512], BF16, kind="ExternalInput")
    wv2 = nc.dram_tensor("wv2", [HR, 512], BF16, kind="ExternalInput")
    ws2 = nc.dram_tensor("ws2", [HR, 512], BF16, kind="ExternalInput")
    colpack = nc.dram_tensor("colpack", [128, CP_W], F32, kind="ExternalInput")
    colpackb = nc.dram_tensor("colpackb", [128, 8], BF16, kind="ExternalInput")
    # bf16 rows: we1(512) we2(512) bvs2(512)
    rowpack = nc.dram_tensor("rowpack", [1, 1536], BF16, kind="ExternalInput")
    # f32 rows: gn2_gamma(512) gn2_beta(512) gn2_ms(512)
    rowpackf = nc.dram_tensor("rowpackf", [1, 1536], F32, kind="ExternalInput")
    # un-normalized h2 block + GraphNorm2 [scl | sh] rows; the affine and the
    # global row-normalize are applied on the host during unsharding
    out = nc.dram_tensor("out", [N, 512], BF16, kind="ExternalOutput")
    gns = nc.dram_tensor("gns", [1, 1024], F32, kind="ExternalOutput")
    if DEBUG:
        dbg_q = nc.dram_tensor("dbg_q", [128, 2048], BF16, kind="ExternalOutput")
        dbg_k = nc.dram_tensor("dbg_k", [128, 4096], BF16, kind="ExternalOutput")
        dbg_v = nc.dram_tensor("dbg_v", [128, 4096], BF16, kind="ExternalOutput")
        dbg_aT = nc.dram_tensor("dbg_aT", [128, 4096], BF16, kind="ExternalOutput")
        dbg_h1sb = nc.dram_tensor("dbg_h1sb", [128, 2048], BF16, kind="ExternalOutput")
        dbg_h1G = nc.dram_tensor("dbg_h1G", [128, 16448], BF16, kind="ExternalOutput")
        dbg_q2 = nc.dram_tensor("dbg_q2", [128, 4096], BF16, kind="ExternalOutput")
        dbg_h2 = nc.dram_tensor("dbg_h2", [128, 4096], BF16, kind="ExternalOutput")
        dbg_sc = nc.dram_tensor("dbg_sc", [128, 32], F32, kind="ExternalOutput")
        dbg_k2 = nc.dram_tensor("dbg_k2", [128, 4096], BF16, kind="ExternalOutput")
        dbg_al = nc.dram_tensor("dbg_al", [128, 1024], F32, kind="ExternalOutput")
        dbg_E = nc.dram_tensor("dbg_E", [128, 1024], BF16, kind="ExternalOutput")

    with tile.TileContext(nc) as tc:
        with (
            tc.tile_pool(name="const", bufs=1) as cp,
            tc.tile_pool(name="xp", bufs=1) as xp,
            tc.tile_pool(name="w2p", bufs=1) as w2p,
            tc.tile_pool(name="sm", bufs=2) as sm,
            tc.tile_pool(name="smo", bufs=2) as smo,
            tc.tile_pool(name="smE", bufs=2) as smE,
            tc.tile_pool(name="smX", bufs=2) as smX,
            tc.tile_pool(name="smc", bufs=1) as smc,
            tc.tile_pool(name="dram", bufs=1, space="DRAM") as dp,
            tc.tile_pool(name="pp", bufs=4, space="PSUM") as pp,
            tc.tile_pool(name="ppt", bufs=2, space="PSUM") as ppt,
            tc.tile_pool(name="pps", bufs=2, space="PSUM") as pps,
        ):
            def load_w(wt, n_k, nm, pool, g=4):
                t = pool.tile([128, n_k * 512], BF16, name=nm)
                for q in range(n_k // g):
                    nc.sync.dma_start(
                        t[:, q * g * 512:(q + 1) * g * 512]
                        .rearrange("p (k n) -> p k n", n=512),
                        wt[q * g * 128:(q + 1) * g * 128, :]
                        .rearrange("(k p) n -> p k n", p=128))
                return t

            # small-pack tiles (DMAs issued after the first weight loads so
            # their many tiny packets don't delay the first matmuls)
            cpk = cp.tile([128, CP_W], F32, name="cpk")
            cpkb = cp.tile([128, 8], BF16, name="cpkb")
            rpk = cp.tile([1, 1536], BF16, name="rpk")
            rpkf = cp.tile([1, 1536], F32, name="rpkf")

            def load_small_packs():
                nc.sync.dma_start(cpk[:, :], colpack[:, :])
                nc.sync.dma_start(cpkb[:, :], colpackb[:, :])
                nc.sync.dma_start(rpk[0:1, :], rowpack[0:1, :])
                nc.sync.dma_start(rpkf[0:1, :], rowpackf[0:1, :])

            def cpc(nm):
                o, w = _CP[nm]
                return cpk[:, o:o + w]

            we1cb = cpkb[:, 0:4]
            we2cb = cpkb[:, 4:8]
            we1r = rpk[0:1, 0:512]
            we2r = rpk[0:1, 512:1024]
            bvs2r = rpk[0:1, 1024:1536]
            gn2g_r = rpkf[0:1, 0:512]
            gn2b_r = rpkf[0:1, 512:1024]
            gn2m_r = rpkf[0:1, 1024:1536]

            ident = cp.tile([128, 128], F32, name="ident")
            make_identity(nc, ident[:, :])
            identb = cp.tile([128, 128], BF16, name="identb")
            nc.vector.tensor_copy(identb[:, :], ident[:, :])
            # 1/N column: GraphNorm2 stat matmuls produce means directly
            ones_col = cp.tile([128, 1], BF16, name="ones_col")
            nc.gpsimd.memset(ones_col[:, :], 1.0 / N)
            ones_row = cp.tile([1, 128], BF16, name="ones_row")
            nc.gpsimd.memset(ones_row[0:1, :], 1.0)
            eps_col = cp.tile([128, 1], F32, name="eps_col")
            nc.gpsimd.memset(eps_col[:, :], EPS)

            ag_in = dp.tile([128, 2056], BF16, name="ag_in")
            ag_out = dp.tile([8, 128, 2056], BF16, name="ag_out", addr_space="Shared")

            # xT stays resident through conv2 (edge terms)
            xTb = xp.tile([128, 8 * N], BF16, name="xTb")

            def gnorm_coeffs(S1t, S2t, gc, bc, mc, P, w, nm):
                def t(name):
                    return smc.tile([P, w], F32, name=f"{name}{nm}")[0:P, :]
                mu = t("mu")
                nc.vector.tensor_scalar_mul(mu, S1t, 1.0 / N)
                ex2 = t("ex2")
                nc.vector.tensor_scalar_mul(ex2, S2t, 1.0 / N)
                msmu = t("msmu")
                nc.vector.tensor_tensor(msmu, mc, mu, ALU.mult)
                tmp = t("tmp")
                nc.vector.tensor_scalar_mul(tmp, mu, 2.0)
                nc.vector.tensor_tensor(tmp, tmp, msmu, ALU.subtract)
                nc.vector.tensor_tensor(tmp, msmu, tmp, ALU.mult)
                var = t("var")
                nc.vector.tensor_tensor(var, ex2, tmp, ALU.subtract)
                nc.scalar.activation(var, var, AF.Sqrt, bias=eps_col[0:P, 0:1])
                rstd = t("rstd")
                nc.vector.reciprocal(rstd, var)
                scl = t("scl")
                nc.vector.tensor_tensor(scl, gc, rstd, ALU.mult)
                sh = t("sh")
                nc.vector.tensor_tensor(sh, scl, msmu, ALU.mult)
                nc.vector.tensor_tensor(sh, bc, sh, ALU.subtract)
                return scl, sh

            # ================= CONV1 =================
            with tc.tile_pool(name="c1", bufs=1) as c1p:
                # loads ordered by first use: q-proj -> k -> v -> attn -> skip
                xtb = c1p.tile([128, 8 * 512], BF16, name="xtb")
                for q in range(2):
                    nc.sync.dma_start(
                        xtb[:, q * 2048:(q + 1) * 2048]
                        .rearrange("p (k n) -> p k n", n=512),
                        xt_tgt[q * 512:(q + 1) * 512, :]
                        .rearrange("(k p) n -> p k n", p=128))
                w1q = load_w(wq1, 8, "w1q", c1p)
                load_small_packs()
                for q in range(4):
                    nc.sync.dma_start(
                        xTb[:, q * 2 * N:(q + 1) * 2 * N]
                        .rearrange("p (k n) -> p k n", n=N),
                        xT[q * 256:(q + 1) * 256, :]
                        .rearrange("(k p) n -> p k n", p=128))
                w1k = load_w(wk1, 8, "w1k", c1p)
                w1v = load_w(wv1, 8, "w1v", c1p)
                xeb = c1p.tile([128, 4 * N], BF16, name="xeb")
                nc.sync.dma_start(
                    xeb[:, :].rearrange("p (k n) -> p k n", n=N),
                    x_edge.rearrange("(k p) n -> p k n", p=128))
                w1s = load_w(ws1, 8, "w1s", c1p)
                # conv2 q/k weights prefetch (stream in during conv1)
                w2q = load_w(wq2, 16, "w2q", w2p)
                w2k = load_w(wk2, 16, "w2k", w2p)

                qTb = c1p.tile([128, 2048], BF16, name="qTb")
                kTb = c1p.tile([128, 4 * N], BF16, name="kTb")
                v1b = c1p.tile([128, 8 * 512], BF16, name="v1b")
                aT1b = c1p.tile([128, 8 * 512], BF16, name="aT1b")
                # agb = AllGather payload: h1 block (cols 0..2047) + S1/S2 cols
                agb = c1p.tile([128, 2056], BF16, name="agb")

                # qT [512d, 512c]
                for dc in range(4):
                    ps = pp.tile([128, 512], F32, name=f"psq{dc}", tag="mm")
                    for fc in range(8):
                        nc.tensor.matmul(
                            ps[:, :],
                            w1q[:, fc * 512 + dc * 128:fc * 512 + dc * 128 + 128],
                            xtb[:, fc * 512:(fc + 1) * 512],
                            start=(fc == 0), stop=(fc == 7))
                    nc.vector.tensor_scalar(qTb[:, dc * 512:(dc + 1) * 512], ps[:, :],
                                            cpc("bq1")[:, dc:dc + 1], None, ALU.add)
                # kT [512d, 1024r]
                for dc in range(4):
                    for rh in range(2):
                        ps = pp.tile([128, 512], F32, name=f"psk{dc}{rh}", tag="mm")
                        for fc in range(8):
                            nc.tensor.matmul(
                                ps[:, :],
                                w1k[:, fc * 512 + dc * 128:fc * 512 + dc * 128 + 128],
                                xTb[:, fc * N + rh * 512:fc * N + (rh + 1) * 512],
                                start=(fc == 0), stop=(fc == 7))
                        nc.vector.tensor_scalar(
                            kTb[:, dc * N + rh * 512:dc * N + (rh + 1) * 512],
                            ps[:, :], cpc("bk1")[:, dc:dc + 1], None, ALU.add)
                # v natural [1024r, 512d]
                for nk in range(8):
                    ps = pp.tile([128, 512], F32, name=f"psv{nk}", tag="mm")
                    for fc in range(8):
                        nc.tensor.matmul(
                            ps[:, :],
                            xTb[:, fc * N + nk * 128:fc * N + nk * 128 + 128],
                            w1v[:, fc * 512:(fc + 1) * 512],
                            start=(fc == 0), stop=(fc == 7))
                    if nk % 2 == 0:
                        nc.vector.tensor_copy(v1b[:, nk * 512:(nk + 1) * 512], ps[:, :])
                    else:
                        nc.scalar.activation(v1b[:, nk * 512:(nk + 1) * 512],
                                             ps[:, :], AF.Copy)

                # qe[c] = q_c . We
                qe_cols = smc.tile([128, 4], F32, name="qe_cols")
                for cc in range(4):
                    psq = pps.tile([128, 1], F32, name=f"psqe{cc}", tag="sm")
                    for dc in range(4):
                        nc.tensor.matmul(
                            psq[:, :],
                            qTb[:, dc * 512 + cc * 128:dc * 512 + cc * 128 + 128],
                            we1cb[:, dc:dc + 1],
                            start=(dc == 0), stop=(dc == 3))
                    nc.scalar.activation(qe_cols[:, cc:cc + 1], psq[:, :], AF.Copy)

                # softmax per 128-target chunk (no max-subtract); the skip
                # projection for feature chunk dc=cc is emitted between qk
                # and the softmax-dependent tensor ops so the tensor engine
                # stays busy during Exp/vector work.
                skb = c1p.tile([128, 2048], BF16, name="skb")
                t1_cols = smc.tile([128, 4], F32, name="t1_cols")
                for cc in range(4):
                    ps0 = pp.tile([128, 512], F32, name=f"psa{cc}", tag="mm")
                    ps1 = pp.tile([128, 512], F32, name=f"psb{cc}", tag="mm")
                    for dc in range(4):
                        nc.tensor.matmul(
                            ps0[:, :],
                            qTb[:, dc * 512 + cc * 128:dc * 512 + cc * 128 + 128],
                            kTb[:, dc * N:dc * N + 512],
                            start=(dc == 0), stop=(dc == 3))
                    for dc in range(4):
                        nc.tensor.matmul(
                            ps1[:, :],
                            qTb[:, dc * 512 + cc * 128:dc * 512 + cc * 128 + 128],
                            kTb[:, dc * N + 512:dc * N + 1024],
                            start=(dc == 0), stop=(dc == 3))
                    ps_sk = pp.tile([128, 512], F32, name=f"pssk{cc}", tag="mm")
                    for fc in range(8):
                        nc.tensor.matmul(
                            ps_sk[:, :],
                            w1s[:, fc * 512 + cc * 128:fc * 512 + cc * 128 + 128],
                            xtb[:, fc * 512:(fc + 1) * 512],
                            start=(fc == 0), stop=(fc == 7))
                    nc.scalar.activation(skb[:, cc * 512:(cc + 1) * 512],
                                         ps_sk[:, :], AF.Identity,
                                         bias=cpc("bvs1")[:, cc:cc + 1])
                    xe = xeb[:, cc * N:(cc + 1) * N]
                    al = sm.tile([128, N], F32, name=f"al{cc}", tag="al")
                    nc.vector.scalar_tensor_tensor(
                        al[:, 0:512], xe[:, 0:512], qe_cols[:, cc:cc + 1],
                        ps0[:, :], ALU.mult, ALU.add)
                    nc.vector.scalar_tensor_tensor(
                        al[:, 512:1024], xe[:, 512:1024], qe_cols[:, cc:cc + 1],
                        ps1[:, :], ALU.mult, ALU.add)
                    Eb = smE.tile([128, N], BF16, name=f"E{cc}", tag="E")
                    scol = smc.tile([128, 1], F32, name=f"s{cc}")
                    nc.scalar.activation(Eb[:, :], al[:, :], AF.Exp,
                                         scale=float(INV_S), accum_out=scol[:, :])
                    escr = smX.tile([128, N], BF16, name="escr", tag="esc")
                    ucol = smc.tile([128, 1], F32, name=f"u{cc}")
                    nc.vector.scalar_tensor_tensor(
                        escr[:, :], Eb[:, :], 1.0, xe[:, :],
                        ALU.mult, ALU.mult, accum_out=ucol[:, :])
                    rcol = smc.tile([128, 1], F32, name=f"r{cc}")
                    nc.vector.reciprocal(rcol[:, :], scol[:, :])
                    nc.vector.tensor_tensor(t1_cols[:, cc:cc + 1], ucol[:, :],
                                            rcol[:, :], ALU.mult)
                    nc.vector.tensor_scalar_mul(Eb[:, :], Eb[:, :], rcol[:, :])
                    for rc in range(8):
                        pst = ppt.tile([128, 128], BF16, name=f"pt{cc}{rc}", tag="tr")
                        nc.tensor.transpose(pst[:, :], Eb[:, rc * 128:(rc + 1) * 128],
                                            identb[:, :])
                        dst = aT1b[:, rc * 512 + cc * 128:rc * 512 + cc * 128 + 128]
                        if rc % 2 == 0:
                            nc.vector.tensor_copy(dst, pst[:, :])
                        else:
                            nc.scalar.activation(dst, pst[:, :], AF.Copy)
                # t1 row [1, 512] bf16
                t1r = smc.tile([1, 512], BF16, name="t1r")
                pstr = pp.tile([1, 512], F32, name="pst1r", tag="mm")
                for cc in range(4):
                    nc.tensor.transpose(pstr[0:1, cc * 128:(cc + 1) * 128],
                                        t1_cols[:, cc:cc + 1], ident[:, :])
                nc.scalar.activation(t1r[0:1, :], pstr[0:1, :], AF.Copy)

                # h1 block [512f, 512c] = attention + edge term, combined with
                # the pre-computed skip block; fused GraphNorm1 partial sums
                S1c = smc.tile([128, 4], F32, name="S1c")
                S2c = smc.tile([128, 4], F32, name="S2c")
                for dc in range(4):
                    ps = pp.tile([128, 512], F32, name=f"pso{dc}", tag="mm")
                    for rc in range(8):
                        nc.tensor.matmul(
                            ps[:, :],
                            v1b[:, rc * 512 + dc * 128:rc * 512 + dc * 128 + 128],
                            aT1b[:, rc * 512:(rc + 1) * 512],
                            start=(rc == 0), stop=False)
                    nc.tensor.matmul(ps[:, :], we1r[0:1, dc * 128:(dc + 1) * 128],
                                     t1r[0:1, :], start=False, stop=True)
                    nc.vector.scalar_tensor_tensor(
                        agb[:, dc * 512:(dc + 1) * 512], ps[:, :], 1.0,
                        skb[:, dc * 512:(dc + 1) * 512], ALU.mult, ALU.add,
                        accum_out=S1c[:, dc:dc + 1])
                    sq = smX.tile([128, 512], BF16, name=f"sq1_{dc}", tag="esc")
                    nc.scalar.activation(sq[:, :], agb[:, dc * 512:(dc + 1) * 512],
                                         AF.Square, accum_out=S2c[:, dc:dc + 1])
                    # stream each finished 1KB-per-partition column block out
                    # on its own queue as it lands; the collective waits all 4
                    if dc < 3:
                        nc.sync.dma_start(ag_in[:, dc * 512:(dc + 1) * 512],
                                          agb[:, dc * 512:(dc + 1) * 512])

                nc.vector.tensor_copy(agb[:, 2048:2052], S1c[:, :])
                nc.vector.tensor_copy(agb[:, 2052:2056], S2c[:, :])
                nc.sync.dma_start(ag_in[:, 1536:2056], agb[:, 1536:2056])
                if DEBUG:
                    nc.sync.dma_start(dbg_q[:, :], qTb[:, :])
                    nc.sync.dma_start(dbg_k[:, :], kTb[:, :])
                    nc.sync.dma_start(dbg_v[:, :], v1b[:, :])
                    nc.sync.dma_start(dbg_aT[:, :], aT1b[:, :])
                    nc.sync.dma_start(dbg_h1sb[:, :], agb[:, 0:2048])

            # ================= CONV2 scope =================
            with (
                tc.tile_pool(name="c2", bufs=1) as c2p,
                tc.tile_pool(name="atp", bufs=2) as atp,
            ):
                # v/skip weights: SBUF freed by conv1, loads overlap the AG
                w2v = load_w(wv2, 16, "w2v", c2p)
                w2s = load_w(ws2, 16, "w2s", c2p)

                # full h1, group-major: group j = core (head j//2, node-half
                # j%2) occupies cols [j*2056, j*2056+2048) = h1 block
                # [128f-part, 4dc*512n], plus cols +2048..+2056 = that core's
                # GraphNorm1 S1/S2 partial columns.
                h1Gb = c2p.tile([128, 8 * 2056], BF16, name="h1Gb")

                def h1s(fc, ch):
                    o = ((fc // 4) * 2 + ch) * 2056 + (fc % 4) * 512
                    return h1Gb[:, o:o + 512]

                def h1c(fc, nk):
                    o = ((fc // 4) * 2 + nk // 4) * 2056 + (fc % 4) * 512 \
                        + (nk % 4) * 128
                    return h1Gb[:, o:o + 128]

                nc.gpsimd.collective_compute(
                    "AllGather", ALU.bypass,
                    ins=[ag_in.opt()], outs=[ag_out.opt()],
                    replica_groups=[list(range(N_CORES))],
                )

                # assemble full h1 (bf16, group-major): contiguous DMAs per
                # peer block (stats ride along), column-split across the sync
                # and scalar engines' DMA queues to halve the drain latency
                for j in range(8):
                    nc.sync.dma_start(
                        h1Gb[:, j * 2056:j * 2056 + 1028],
                        ag_out[j, :, 0:1028])
                    nc.scalar.dma_start(
                        h1Gb[:, j * 2056 + 1028:(j + 1) * 2056],
                        ag_out[j, :, 1028:2056])
                s1r = [h1Gb[:, j * 2056 + 2048:j * 2056 + 2052] for j in range(8)]
                s2r = [h1Gb[:, j * 2056 + 2052:j * 2056 + 2056] for j in range(8)]
                S1a = smc.tile([128, 16], F32, name="S1a")
                S2a = smc.tile([128, 16], F32, name="S2a")
                for h in range(4):
                    nc.vector.tensor_tensor(S1a[:, 4 * h:4 * h + 4], s1r[2 * h],
                                            s1r[2 * h + 1], ALU.add)
                    nc.vector.tensor_tensor(S2a[:, 4 * h:4 * h + 4], s2r[2 * h],
                                            s2r[2 * h + 1], ALU.add)

                if DEBUG:
                    nc.sync.dma_start(dbg_h1G[:, :], h1Gb[:, :])
                scl1, sh1 = gnorm_coeffs(S1a[:, :], S2a[:, :], cpc("gn1g"),
                                         cpc("gn1b"), cpc("gn1m"), 128, 16, "g1")
                eng = 0
                for g in range(2):
                    for f in range(16):
                        o = ((f // 4) * 2 + g) * 2056 + (f % 4) * 512
                        dst = h1Gb[:, o:o + 512]
                        if eng % 3 == 0:
                            nc.vector.tensor_scalar(dst, dst, scl1[:, f:f + 1],
                                                    sh1[:, f:f + 1], ALU.mult, ALU.add)
                        elif eng % 3 == 1:
                            nc.scalar.activation(dst, dst, AF.Identity,
                                                 bias=sh1[:, f:f + 1],
                                                 scale=scl1[:, f:f + 1])
                        else:
                            nc.gpsimd.tensor_scalar(dst, dst, scl1[:, f:f + 1],
                                                    sh1[:, f:f + 1], ALU.mult, ALU.add)
                        eng += 1

                # ---- conv2 compute (natural-layout output) ----
                q2Tb = c2p.tile([128, 4 * N], BF16, name="q2Tb")
                k2Tb = c2p.tile([128, 4 * N], BF16, name="k2Tb")
                v2b = c2p.tile([128, 8 * 512], BF16, name="v2b")
                h2nb = c2p.tile([128, 8 * 512], BF16, name="h2nb")
                rcols = smc.tile([128, 8], F32, name="rcols")

                for dc in range(4):
                    for ch in range(2):
                        ps = pp.tile([128, 512], F32, name=f"ps2q{dc}{ch}", tag="mm")
                        for fc in range(16):
                            nc.tensor.matmul(
                                ps[:, :],
                                w2q[:, fc * 512 + dc * 128:fc * 512 + dc * 128 + 128],
                                h1s(fc, ch), start=(fc == 0), stop=(fc == 15))
                        nc.vector.tensor_scalar(
                            q2Tb[:, dc * N + ch * 512:dc * N + (ch + 1) * 512],
                            ps[:, :], cpc("bq2")[:, dc:dc + 1], None, ALU.add)
                for dc in range(4):
                    for ch in range(2):
                        ps = pp.tile([128, 512], F32, name=f"ps2k{dc}{ch}", tag="mm")
                        for fc in range(16):
                            nc.tensor.matmul(
                                ps[:, :],
                                w2k[:, fc * 512 + dc * 128:fc * 512 + dc * 128 + 128],
                                h1s(fc, ch), start=(fc == 0), stop=(fc == 15))
                        nc.vector.tensor_scalar(
                            k2Tb[:, dc * N + ch * 512:dc * N + (ch + 1) * 512],
                            ps[:, :], cpc("bk2")[:, dc:dc + 1], None, ALU.add)
                if DEBUG:
                    nc.sync.dma_start(dbg_k2[:, :], k2Tb[:, :])
                for nk in range(8):
                    ps = pp.tile([128, 512], F32, name=f"ps2v{nk}", tag="mm")
                    for fc in range(16):
                        nc.tensor.matmul(ps[:, :], h1c(fc, nk),
                                         w2v[:, fc * 512:(fc + 1) * 512],
                                         start=(fc == 0), stop=(fc == 15))
                    if nk % 2 == 0:
                        nc.vector.tensor_copy(v2b[:, nk * 512:(nk + 1) * 512], ps[:, :])
                    else:
                        nc.scalar.activation(v2b[:, nk * 512:(nk + 1) * 512],
                                             ps[:, :], AF.Copy)

                qe2 = smc.tile([128, 8], F32, name="qe2")
                for cc in range(8):
                    psq = pps.tile([128, 1], F32, name=f"ps2e{cc}", tag="sm")
                    for dc in range(4):
                        nc.tensor.matmul(
                            psq[:, :],
                            q2Tb[:, dc * N + cc * 128:dc * N + cc * 128 + 128],
                            we2cb[:, dc:dc + 1], start=(dc == 0), stop=(dc == 3))
                    nc.scalar.activation(qe2[:, cc:cc + 1], psq[:, :], AF.Copy)

                if DEBUG:
                    nc.sync.dma_start(dbg_q2[:, :], q2Tb[:, :])
                if DEBUG:
                    nc.sync.dma_start(dbg_sc[:, 8:16], qe2[:, :])
                t2rr = smc.tile([1, N], BF16, name="t2rr")
                scols_dbg = smc.tile([128, 8], F32, name="scols_dbg")

                sq2b = c2p.tile([128, 8 * 512], BF16, name="sq2b")
                for cc in range(8):
                    ps0 = pp.tile([128, 512], F32, name=f"p2a{cc}", tag="mm")
                    ps1 = pp.tile([128, 512], F32, name=f"p2b{cc}", tag="mm")
                    for dc in range(4):
                        nc.tensor.matmul(
                            ps0[:, :],
                            q2Tb[:, dc * N + cc * 128:dc * N + cc * 128 + 128],
                            k2Tb[:, dc * N:dc * N + 512],
                            start=(dc == 0), stop=(dc == 3))
                    for dc in range(4):
                        nc.tensor.matmul(
                            ps1[:, :],
                            q2Tb[:, dc * N + cc * 128:dc * N + cc * 128 + 128],
                            k2Tb[:, dc * N + 512:dc * N + 1024],
                            start=(dc == 0), stop=(dc == 3))
                    # skip projection emitted here: fills the tensor engine
                    # while the softmax chain runs on vector/scalar
                    ps_skp = pp.tile([128, 512], F32, name=f"psk2{cc}", tag="mm")
                    for fc in range(16):
                        nc.tensor.matmul(ps_skp[:, :], h1c(fc, cc),
                                         w2s[:, fc * 512:(fc + 1) * 512],
                                         start=(fc == 0), stop=False)
                    nc.tensor.matmul(ps_skp[:, :], ones_row[0:1, :], bvs2r[0:1, :],
                                     start=False, stop=True)
                    skp_sb = smo.tile([128, 512], F32, name=f"skp{cc}", tag="of")
                    nc.scalar.activation(skp_sb[:, :], ps_skp[:, :], AF.Copy)
                    xe = xTb[:, cc * N:(cc + 1) * N]
                    al = sm.tile([128, N], F32, name=f"al2_{cc}", tag="al")
                    nc.vector.scalar_tensor_tensor(
                        al[:, 0:512], xe[:, 0:512], qe2[:, cc:cc + 1],
                        ps0[:, :], ALU.mult, ALU.add)
                    nc.vector.scalar_tensor_tensor(
                        al[:, 512:1024], xe[:, 512:1024], qe2[:, cc:cc + 1],
                        ps1[:, :], ALU.mult, ALU.add)
                    Eb = smE.tile([128, N], BF16, name=f"E2_{cc}", tag="E")
                    scol = smc.tile([128, 1], F32, name=f"s2_{cc}")
                    nc.scalar.activation(Eb[:, :], al[:, :], AF.Exp,
                                         scale=float(INV_S), accum_out=scol[:, :])
                    escr = smX.tile([128, N], BF16, name="escr2", tag="esc")
                    ucol = smc.tile([128, 1], F32, name=f"u2_{cc}")
                    nc.vector.scalar_tensor_tensor(
                        escr[:, :], Eb[:, :], 1.0, xe[:, :],
                        ALU.mult, ALU.mult, accum_out=ucol[:, :])
                    nc.vector.reciprocal(rcols[:, cc:cc + 1], scol[:, :])
                    if DEBUG:
                        nc.vector.tensor_copy(scols_dbg[:, cc:cc + 1], scol[:, :])
                        if cc == 0:
                            nc.sync.dma_start(dbg_al[:, :], al[:, :])
                            nc.sync.dma_start(dbg_E[:, :], Eb[:, :])
                    pstt = pps.tile([1, 128], F32, name=f"ptt{cc}", tag="sm")
                    nc.tensor.transpose(pstt[0:1, :], ucol[:, :], ident[:, :])
                    nc.scalar.activation(t2rr[0:1, cc * 128:(cc + 1) * 128],
                                         pstt[0:1, :], AF.Copy)
                    # transpose E into the per-chunk aT ring
                    aTc = atp.tile([128, 8 * 128], BF16, name=f"aTc{cc}", tag="at")
                    for rc in range(8):
                        pst = ppt.tile([128, 128], BF16, name=f"p2t{cc}{rc}", tag="tr")
                        nc.tensor.transpose(pst[:, :], Eb[:, rc * 128:(rc + 1) * 128],
                                            identb[:, :])
                        dst = aTc[:, rc * 128:(rc + 1) * 128]
                        if rc % 2 == 0:
                            nc.vector.tensor_copy(dst, pst[:, :])
                        else:
                            nc.scalar.activation(dst, pst[:, :], AF.Copy)

                    # output chunk cc: natural layout [128c, 512d]
                    ps_att = pp.tile([128, 512], F32, name=f"pat{cc}", tag="mm")
                    for rc in range(8):
                        nc.tensor.matmul(
                            ps_att[:, :], aTc[:, rc * 128:(rc + 1) * 128],
                            v2b[:, rc * 512:(rc + 1) * 512],
                            start=(rc == 0), stop=False)
                    nc.tensor.matmul(ps_att[:, :],
                                     t2rr[0:1, cc * 128:(cc + 1) * 128],
                                     we2r[0:1, :], start=False, stop=True)
                    nc.vector.scalar_tensor_tensor(
                        h2nb[:, cc * 512:(cc + 1) * 512], ps_att[:, :],
                        rcols[:, cc:cc + 1], skp_sb[:, :], ALU.mult, ALU.add)
                    # stream the raw h2 chunk out now (hidden under the loop)
                    nc.sync.dma_start(out[cc * 128:(cc + 1) * 128, :],
                                      h2nb[:, cc * 512:(cc + 1) * 512])
                    # square for GraphNorm2 E[x^2], computed as chunks land
                    sqd = sq2b[:, cc * 512:(cc + 1) * 512]
                    if cc % 2 == 0:
                        nc.gpsimd.tensor_tensor(
                            sqd, h2nb[:, cc * 512:(cc + 1) * 512],
                            h2nb[:, cc * 512:(cc + 1) * 512], ALU.mult)
                    else:
                        nc.scalar.activation(sqd,
                                             h2nb[:, cc * 512:(cc + 1) * 512],
                                             AF.Square)

                # GraphNorm2 stats via 1/N-matmuls -> means directly
                S1ps = pp.tile([1, 512], F32, name="S1ps", tag="mm")
                S2ps = pp.tile([1, 512], F32, name="S2ps", tag="mm")
                for cc in range(8):
                    nc.tensor.matmul(S1ps[0:1, :], ones_col[:, 0:1],
                                     h2nb[:, cc * 512:(cc + 1) * 512],
                                     start=(cc == 0), stop=(cc == 7))
                for cc in range(8):
                    nc.tensor.matmul(S2ps[0:1, :], ones_col[:, 0:1],
                                     sq2b[:, cc * 512:(cc + 1) * 512],
                                     start=(cc == 0), stop=(cc == 7))

                # GraphNorm2 coeffs computed directly in row space [1, 512];
                # S1ps/S2ps already hold mu and E[x^2] (1/N-weighted matmul).
                # rA = msmu then sh; rB = tmp -> var -> std -> scl (in-place)
                def rt(name):
                    return smc.tile([1, 512], F32, name=name)[0:1, :]
                rA = rt("rA")
                nc.vector.tensor_tensor(rA, gn2m_r, S1ps[0:1, :], ALU.mult)
                rB = rt("rB")
                nc.vector.scalar_tensor_tensor(
                    rB, S1ps[0:1, :], 2.0, rA, ALU.mult, ALU.subtract)
                nc.vector.tensor_tensor(rB, rA, rB, ALU.mult)
                nc.vector.tensor_tensor(rB, S2ps[0:1, :], rB, ALU.subtract)
                nc.scalar.activation(rB, rB, AF.Sqrt, bias=eps_col[0:1, 0:1])
                rC = rt("rC")
                nc.vector.reciprocal(rC, rB)
                nc.vector.tensor_tensor(rB, gn2g_r, rC, ALU.mult)
                nc.vector.tensor_tensor(rA, rB, rA, ALU.mult)
                nc.vector.tensor_tensor(rC, gn2b_r, rA, ALU.subtract)
                nc.sync.dma_start(gns[0:1, 0:512], rB)
                nc.sync.dma_start(gns[0:1, 512:1024], rC)
    return nc


_NC_CACHE = None


def _get_nc():
    global _NC_CACHE
    if _NC_CACHE is None:
        nc = build_nc()
        # local walrus only accepts one sync-wait per CTRL-class instruction
        for f in nc.m.functions:
            for bb in f.blocks:
                changed = False
                new_list = []
                for ins in bb.instructions:
                    si = ins.sync_info
                    if si is not None and len(si.on_wait) > 1:
                        waits = list(si.on_wait)
                        for i, w in enumerate(waits[:-1]):
                            nop = mybir.InstNoOp(
                                name=f"{ins.name}_presplit{i}", engine=ins.engine)
                            nop.sync_info = mybir.SyncInfo(on_wait=[w], on_update=[])
                            new_list.append(nop)
                        ins.sync_info = mybir.SyncInfo(
                            on_wait=[waits[-1]], on_update=list(si.on_update))
                        changed = True
                    new_list.append(ins)
                if changed:
                    bb.instructions = new_list
        _NC_CACHE = nc
    return _NC_CACHE


def _col(v, w):
    """[128*w] vector -> [128, w] column layout (col j = elems j*128..+128)."""
    return np.ascontiguousarray(np.asarray(v, np.float32).reshape(w, 128).T)


def build_in_maps(inputs):
    x = np.asarray(inputs["x"], np.float32)
    bf = ml_dtypes.bfloat16

    def c(a, dt=np.float32):
        return np.ascontiguousarray(a).astype(dt)

    f32 = {k: np.asarray(v, np.float32) for k, v in inputs.items() if k != "x"}
    xT = np.ascontiguousarray(x.T)
    in_maps = []
    for i in range(N_CORES):
        h, g = i // 2, i % 2
        s1, s2i = slice(512 * h, 512 * (h + 1)), slice(512 * i, 512 * (i + 1))
        we1h = f32["e1_w"].reshape(4, 512)[h]
        we2h = f32["e2_w"].reshape(8, 512)[i]
        bvs1 = f32["v1_b"][s1] + f32["s1_b"][s1]
        bvs2 = f32["v2_b"][s2i] + f32["s2_b"][s2i]
        colpack = np.concatenate([
            _col(f32["q1_b"][s1], 4), _col(f32["k1_b"][s1], 4), _col(bvs1, 4),
            _col(f32["gn1_gamma"], 16), _col(f32["gn1_beta"], 16),
            _col(f32["gn1_ms"], 16),
            _col(f32["q2_b"][s2i], 4), _col(f32["k2_b"][s2i], 4),
            _col(f32["gn2_gamma"][s2i], 4), _col(f32["gn2_beta"][s2i], 4),
            _col(f32["gn2_ms"][s2i], 4),
        ], axis=1)
        colpackb = np.concatenate([_col(we1h, 4), _col(we2h, 4)], axis=1)
        rowpack = np.concatenate([we1h, we2h, bvs2]).reshape(1, 1536)
        rowpackf = np.concatenate([
            f32["gn2_gamma"][s2i], f32["gn2_beta"][s2i], f32["gn2_ms"][s2i],
        ]).reshape(1, 1536)
        m = {
            "xT": c(xT, bf),
            "xt_tgt": c(xT[:, 512 * g:512 * (g + 1)], bf),
            "x_edge": c(xT[512 * g:512 * (g + 1), :], bf),
            "wq1": c(f32["q1_w"][:, s1], bf),
            "wk1": c(f32["k1_w"][:, s1], bf),
            "wv1": c(f32["v1_w"][:, s1], bf),
            "ws1": c(f32["s1_w"][:, s1], bf),
            "wq2": c(f32["q2_w"][:, s2i], bf),
            "wk2": c(f32["k2_w"][:, s2i], bf),
            "wv2": c(f32["v2_w"][:, s2i], bf),
            "ws2": c(f32["s2_w"][:, s2i], bf),
            "colpack": c(colpack),
            "colpackb": c(colpackb, bf),
            "rowpack": c(rowpack, bf),
            "rowpackf": c(rowpackf),
        }
        in_maps.append(m)
    return in_maps


def kernel(**inputs):
    in_maps = build_in_maps(inputs)
    res = _run_cached(in_maps)
    full = np.empty((N, 2 * HR), np.float32)
    for i in range(N_CORES):
        g = np.asarray(res[i]["gns"], np.float32).reshape(1024)
        h2 = np.asarray(res[i]["out"], np.float32)
        full[:, 512 * i:512 * (i + 1)] = h2 * g[:512] + g[512:]
    rnsum = (full.astype(np.float64) ** 2).sum(axis=1)
    full /= np.sqrt(rnsum)[:, None].astype(np.float32)
    return full


_RUNNER = None


def _get_runner():
    """Build the sharded jitted executable once per process."""
    global _RUNNER
    if _RUNNER is not None:
        return _RUNNER
    import jax
    from jax.sharding import Mesh, PartitionSpec, NamedSharding
    from jax.experimental.shard_map import shard_map
    from concourse import bass2jax
    from concourse.bass2jax import _bass_exec_p, install_neuronx_cc_hook

    nc = _get_nc()
    install_neuronx_cc_hook()
    partition_name = nc.partition_id_tensor.name if nc.partition_id_tensor else None
    in_names, out_names, out_avals, zero_outs = [], [], [], []
    for alloc in nc.m.functions[0].allocations:
        if not isinstance(alloc, mybir.MemoryLocationSet):
            continue
        name = alloc.memorylocations[0].name
        if alloc.kind == "ExternalInput":
            if name != partition_name:
                in_names.append(name)
        elif alloc.kind == "ExternalOutput":
            out_names.append(name)
            out_avals.append(jax.core.ShapedArray(
                tuple(alloc.tensor_shape), mybir.dt.np(alloc.dtype)))
            zero_outs.append(np.zeros(tuple(alloc.tensor_shape),
                                      mybir.dt.np(alloc.dtype)))
    n_params, n_outs = len(in_names), len(out_avals)
    all_names = in_names + out_names + ([partition_name] if partition_name else [])
    donate = tuple(range(n_params, n_params + n_outs))

    def _body(*args):
        operands = list(args)
        if partition_name is not None:
            operands.append(bass2jax.partition_id_tensor())
        return tuple(_bass_exec_p.bind(
            *operands, out_avals=tuple(out_avals), in_names=tuple(all_names),
            out_names=tuple(out_names), lowering_input_output_aliases=(),
            sim_require_finite=True, sim_require_nnan=True, nc=nc))

    devices = jax.devices()[:N_CORES]
    mesh = Mesh(np.asarray(devices), ("core",))
    sharded = jax.jit(
        shard_map(_body, mesh=mesh,
                  in_specs=(PartitionSpec("core"),) * (n_params + n_outs),
                  out_specs=(PartitionSpec("core"),) * n_outs,
                  check_rep=False),
        donate_argnums=donate, keep_unused=True)
    sh = NamedSharding(mesh, PartitionSpec("core"))
    _RUNNER = (sharded, sh, in_names, out_names, out_avals, zero_outs, jax)
    return _RUNNER


def _run_cached(in_maps):
    sharded, sh, in_names, out_names, out_avals, zero_outs, jax = _get_runner()
    concat_in = [np.concatenate([np.asarray(in_maps[c][nm])
                                 for c in range(N_CORES)], axis=0)
                 for nm in in_names]
    dev_in = [jax.device_put(a, sh) for a in concat_in]
    zs = [jax.device_put(np.zeros((N_CORES * z.shape[0], *z.shape[1:]), z.dtype), sh)
          for z in zero_outs]
    outs = sharded(*dev_in, *zs)
    outs = [np.asarray(o).reshape(N_CORES, *out_avals[i].shape)
            for i, o in enumerate(outs)]
    return [{nm: outs[i][c] for i, nm in enumerate(out_names)}
            for c in range(N_CORES)]

